# revision 11
# baseline (speedup 1.0000x reference)
"""Trainium2 Bass kernel for nn_BinarizedArithmeticModule (8-core SPMD).

Math: out = unbinarize((tanh(W_hat) * sigmoid(M_hat)) @ binarize(inputs))
  inputs [1024] f32 -> bits [32768] {0,1}
  W_hat, M_hat [4096, 32768] f32
  binary_out [4096] f32 -> round/clip -> pack -> out [128] f32

Sharding: W_hat/M_hat row-sharded, 512 rows per core; bits replicated.
Each core computes its 512 partial dot products; host gathers + unbinarizes.
"""

import numpy as np
import ml_dtypes

import concourse.bass as bass
import concourse.bacc as bacc
import concourse.tile as tile
from concourse import mybir
from concourse import bass_utils

IN_BITS = 32768
OUT_BITS = 4096
N_CORES = 8
ROWS_PER_CORE = OUT_BITS // N_CORES  # 512
P = 128
# k-chunk schedule: big 2 MiB tiles for DMA efficiency, tapered tail so the
# last tile's ACT->DVE chain after the final DMA is short.
CHUNKS = [4096] * 7 + [2048, 1024, 1024]
R_CHUNKS = ROWS_PER_CORE // P         # 4

_f32 = mybir.dt.float32
_bf16 = mybir.dt.bfloat16


def build_nc(rows_per_core=ROWS_PER_CORE, chunks=None, bufs_wm=2):
    if chunks is None:
        chunks = CHUNKS
    in_bits = sum(chunks)
    r_chunks = rows_per_core // P
    nkc = len(chunks)
    nc = bacc.Bacc("TRN2", target_bir_lowering=False, debug=False,
                   num_devices=N_CORES)
    wh = nc.dram_tensor("wh", [rows_per_core, in_bits], _f32,
                        kind="ExternalInput").ap()
    mh = nc.dram_tensor("mh", [rows_per_core, in_bits], _f32,
                        kind="ExternalInput").ap()
    bitsd = nc.dram_tensor("bits", [1, in_bits], _bf16,
                           kind="ExternalInput").ap()
    outd = nc.dram_tensor("out", [P, r_chunks], _f32,
                          kind="ExternalOutput").ap()

    with tile.TileContext(nc) as tc:
        with (
            tc.tile_pool(name="wp", bufs=bufs_wm) as wp,
            tc.tile_pool(name="mp", bufs=bufs_wm) as mp,
            tc.tile_pool(name="tp", bufs=2) as tp,
            tc.tile_pool(name="up", bufs=2) as up,
            tc.tile_pool(name="sp", bufs=2) as sp,
            tc.tile_pool(name="dp", bufs=1, space="PSUM") as dp,
            tc.tile_pool(name="bp", bufs=2) as bp,
            tc.tile_pool(name="bcp", bufs=2) as bcp,
            tc.tile_pool(name="accp", bufs=1) as accp,
        ):
            acc = accp.tile([P, r_chunks * nkc], _f32)
            res = accp.tile([P, r_chunks], _f32)
            off = 0
            for k, f in enumerate(chunks):
                ks = slice(off, off + f)
                off += f
                bsb = bp.tile([1, f], _bf16)
                # SWDGE keeps these small loads off the W-load HWDGE ring
                nc.gpsimd.dma_start(bsb[:, :], bitsd[0:1, ks])
                bbc = bcp.tile([P, f], _bf16)
                nc.gpsimd.partition_broadcast(bbc[:, :], bsb[0:1, :])
                for r in range(r_chunks):
                    rs = bass.ts(r, P)
                    w = wp.tile([P, f], _f32)
                    nc.sync.dma_start(w[:, :], wh[rs, ks])
                    m = mp.tile([P, f], _f32)
                    nc.scalar.dma_start(m[:, :], mh[rs, ks])
                    t = tp.tile([P, f], _f32)
                    nc.scalar.activation(t[:, :], w[:, :],
                                         mybir.ActivationFunctionType.Tanh)
                    u = up.tile([P, f], _f32)
                    nc.scalar.activation(u[:, :], m[:, :],
                                         mybir.ActivationFunctionType.Sigmoid)
                    s = sp.tile([P, f], _f32)
                    nc.vector.tensor_tensor(s[:, :], t[:, :], u[:, :],
                                            mybir.AluOpType.mult)
                    d = dp.tile([P, f], _f32)
                    col = r * nkc + k
                    nc.vector.scalar_tensor_tensor(
                        out=d[:, :], in0=s[:, :], scalar=1.0, in1=bbc[:, :],
                        op0=mybir.AluOpType.mult, op1=mybir.AluOpType.mult,
                        accum_out=acc[:, col:col + 1],
                    )
            for r in range(r_chunks):
                nc.vector.reduce_sum(res[:, r:r + 1],
                                     acc[:, r * nkc:(r + 1) * nkc],
                                     axis=mybir.AxisListType.X)
            nc.sync.dma_start(outd[:, :], res[:, :])
    nc.compile()
    return nc


def binarize_np(x: np.ndarray) -> np.ndarray:
    """float32 [N] -> float32 bits [N*32], matching reference binarize_float."""
    x = np.ascontiguousarray(x, dtype=np.float32)
    return np.unpackbits(x.view(np.uint8)).astype(np.float32)


def unbinarize_np(vals: np.ndarray) -> np.ndarray:
    """float [M*32] -> float32 [M], matching reference unbinarize."""
    b = np.clip(np.round(vals), 0.0, 1.0).astype(np.uint8)
    return np.packbits(b).view(np.uint32).view(np.float32)


_NC_CACHE = None


def make_in_maps(inputs, W_hat, M_hat):
    bits = binarize_np(inputs)
    bits_bf = bits.astype(ml_dtypes.bfloat16).reshape(1, IN_BITS)
    W = np.ascontiguousarray(W_hat, dtype=np.float32)
    M = np.ascontiguousarray(M_hat, dtype=np.float32)
    in_maps = []
    for c in range(N_CORES):
        sl = slice(c * ROWS_PER_CORE, (c + 1) * ROWS_PER_CORE)
        in_maps.append({"wh": W[sl], "mh": M[sl], "bits": bits_bf})
    return in_maps


def gather_output(results) -> np.ndarray:
    # out[p, r] holds the partial sum for local row r*128+p
    parts = [np.asarray(results[c]["out"]).T.reshape(-1)
             for c in range(N_CORES)]
    return unbinarize_np(np.concatenate(parts))


def kernel(inputs: np.ndarray, W_hat: np.ndarray, M_hat: np.ndarray,
           **_extra):
    global _NC_CACHE
    if _NC_CACHE is None:
        _NC_CACHE = build_nc()
    nc = _NC_CACHE
    in_maps = make_in_maps(inputs, W_hat, M_hat)
    r = bass_utils.run_bass_kernel_spmd(nc, in_maps,
                                        core_ids=list(range(N_CORES)))
    return gather_output(r.results)
